# revision 5
# baseline (speedup 1.0000x reference)
"""Trainium2 Bass kernel: y = LP(square(BP(x))) cascaded-biquad IIR filtering.

x: [16, 16384, 64] fp32; bp_sos/lp_sos: [2, 6] second-order sections.
Reference applies, per (batch, channel) sequence along time:
  w = sosfilt(w, bp_sos); w = sosfilt(w*w, lp_sos)
with zero initial conditions (Direct Form I biquads).

Strategy (exact FIR reformulation, no recurrence on device):
  Each 2-biquad cascade is an order-4 IIR whose impulse response h decays
  below 1e-10 (relative) within 128 taps, so the filter is numerically a
  256-tap FIR:  y[t] = sum_{i<256} h[i] x[t-i].
  With the per-core layout Xm[l, c*128+s] (chunk position l on the
  partition axis, (chunk, sequence) on the free axis), each output column
  tile is two accumulating Toeplitz matmuls:
     y_c = T0 @ x_c + T1 @ x_{c-1}
  where T0[i,j] = h[i-j], T1[i,j] = h[128+i-j].  x_{c-1} is just the same
  SBUF buffer read 128 columns to the left (zero-padded at the start), so
  there is no state, no scan, and no shuffle traffic.

  The whole device pipeline runs in fp16 (inputs, weights, intermediates,
  outputs) with fp32 PSUM accumulation; host-measured end-to-end error vs
  the float64 reference is ~1e-3 relative (gate is 2e-2).  HBM traffic is
  halved vs fp32.

Sharding: data-parallel over the 1024 = 16*64 independent sequences;
128 sequences per core.  Host does the (cheap) layout transposes +
fp32<->fp16 casts; device does all filtering math.
"""

import numpy as np

# ---------------------------------------------------------------- constants
B, T, C = 16, 16384, 64
NCORES = 8
L = 128           # chunk length (partition dim)
N = T // L        # 128 chunks per sequence
S = 128           # sequences per core
TILE = 512        # matmul moving free-dim (one PSUM bank of fp32)
NT = (N * S) // TILE  # 32 column tiles


def _combine_sos(sos):
    """[2,6] sos -> normalized order-4 (b[0..4], a[0..4]) float64, a[0]=1."""
    sos = np.asarray(sos, dtype=np.float64)
    b1, a1 = sos[0, :3] / sos[0, 3], sos[0, 3:] / sos[0, 3]
    b2, a2 = sos[1, :3] / sos[1, 3], sos[1, 3:] / sos[1, 3]
    return np.convolve(b1, b2), np.convolve(a1, a2)


def _impulse(b, a, n):
    """Impulse response h[0..n-1] of the order-4 filter (b, a), float64."""
    u = np.zeros(n + 4)
    y = np.zeros(n + 4)
    u[4] = 1.0
    for t in range(n):
        acc = b[0] * u[t + 4] + b[1] * u[t + 3] + b[2] * u[t + 2] \
            + b[3] * u[t + 1] + b[4] * u[t]
        acc -= a[1] * y[t + 3] + a[2] * y[t + 2] + a[3] * y[t + 1] + a[4] * y[t]
        y[t + 4] = acc
    return y[4:]


def _toeplitz_lhsT(h, k):
    """lhsT for block k: lhsT[j, i] = h[128k + i - j]  (so out = Tk @ x)."""
    idx = np.arange(L)
    lag = 128 * k + idx[None, :] - idx[:, None]   # [j, i]
    m = (lag >= 0) & (lag < len(h))
    out = np.zeros((L, L))
    out[m] = h[np.clip(lag, 0, len(h) - 1)][m]
    return out


def _filter_weights(sos):
    """Per-filter (lhsT_T0, lhsT_T1) fp16 weight matrices + truncation err."""
    b, a = _combine_sos(sos)
    h = _impulse(b, a, 512)
    trunc = np.abs(h[256:]).max() / np.abs(h).max()
    return (_toeplitz_lhsT(h, 0).astype(np.float16),
            _toeplitz_lhsT(h, 1).astype(np.float16)), trunc


# ---------------------------------------------------------------- device IR
_PROGRAM_CACHE = {}


def _build_program():
    import concourse.bass as bass
    import concourse.mybir as mybir
    import concourse.tile as tile
    from concourse import bacc

    F32 = mybir.dt.float32
    F16 = mybir.dt.float16
    ts = bass.ts
    PAD = L          # leading zero columns (one chunk) in SBUF buffers
    W = PAD + T      # SBUF buffer width

    nc = bacc.Bacc(None)
    x_d = nc.declare_dram_parameter("x", [128, T], F16, isOutput=False)
    w_d = {f: nc.declare_dram_parameter(f, [128, 128], F16, False)
           for f in ("t0bp", "t1bp", "t0lp", "t1lp")}
    out_d = nc.declare_dram_parameter("out", [128, T], F16, isOutput=True)

    with tile.TileContext(nc) as tc:
        with (
            tc.tile_pool(name="big", bufs=1) as bigpool,
            tc.tile_pool(name="consts", bufs=1) as cpool,
            tc.tile_pool(name="psA", bufs=2, space=bass.MemorySpace.PSUM) as psA,
            tc.tile_pool(name="psB", bufs=2, space=bass.MemorySpace.PSUM) as psB,
        ):
            xb = bigpool.tile([128, W], F16, tag="xb", name="xb")
            y1 = bigpool.tile([128, W], F16, tag="y1", name="y1")
            ob = bigpool.tile([128, T], F16, tag="ob", name="ob")
            wt = {f: cpool.tile([128, 128], F16, tag=f, name=f)
                  for f in ("t0bp", "t1bp", "t0lp", "t1lp")}
            for f in wt:
                nc.sync.dma_start(out=wt[f][:], in_=w_d[f][:])
            nc.vector.memzero(xb[:, 0:PAD])
            nc.gpsimd.memzero(y1[:, 0:PAD])
            NCH = 16                      # input DMA chunks
            for g in range(NCH):
                nc.sync.dma_start(out=xb[:, PAD + g * (T // NCH):
                                         PAD + (g + 1) * (T // NCH)],
                                  in_=x_d[:, ts(g, T // NCH)])

            def emit_pair(pt, IN, OUT, t0, t1, pool, square):
                """Two 512-col tiles (2pt, 2pt+1) into one 2-bank PSUM tile:
                T0 then T1 accumulate, then a single 1024-wide drain."""
                ps = pool.tile([128, 2 * TILE], F32, tag="ps", name="ps")
                for j in range(2):
                    t = 2 * pt + j
                    nc.tensor.matmul(ps[:, ts(j, TILE)], t0[:],
                                     IN[:, PAD + t * TILE: PAD + (t + 1) * TILE],
                                     start=True, stop=False,
                                     skip_group_check=True)
                for j in range(2):
                    t = 2 * pt + j
                    nc.tensor.matmul(ps[:, ts(j, TILE)], t1[:],
                                     IN[:, t * TILE: (t + 1) * TILE],
                                     start=False, stop=True,
                                     skip_group_check=True)
                base = 2 * pt * TILE
                if square:
                    # single-input Square: only Activation can read PSUM once
                    nc.scalar.square(OUT[:, PAD + base: PAD + base + 2 * TILE],
                                     ps[:])
                else:
                    # gpsimd cannot access PSUM on trn2 -> all copies on DVE
                    nc.vector.tensor_copy(OUT[:, base: base + 2 * TILE], ps[:])

            NP = NT // 2                 # 16 tile-pairs per filter
            emit_pair(0, xb, y1, wt["t0bp"], wt["t1bp"], psA, True)
            for pt in range(1, NP):
                emit_pair(pt, xb, y1, wt["t0bp"], wt["t1bp"], psA, True)
                emit_pair(pt - 1, y1, ob, wt["t0lp"], wt["t1lp"], psB, False)
                if pt >= 2:
                    g = pt - 2
                    nc.scalar.dma_start(out=out_d[:, ts(g, T // NCH)],
                                        in_=ob[:, ts(g, T // NCH)])
            emit_pair(NP - 1, y1, ob, wt["t0lp"], wt["t1lp"], psB, False)
            for g in range(NCH - 2, NCH):
                nc.scalar.dma_start(out=out_d[:, ts(g, T // NCH)],
                                    in_=ob[:, ts(g, T // NCH)])

    nc.compile()
    return nc


def _get_program():
    if "p" not in _PROGRAM_CACHE:
        _PROGRAM_CACHE["p"] = _build_program()
    return _PROGRAM_CACHE["p"]


# ---------------------------------------------------------------- host entry
def _shard_inputs(x):
    """x [B,T,C] -> list of per-core Xm [128, T] fp16 arrays."""
    xs = np.ascontiguousarray(np.transpose(np.asarray(x, dtype=np.float32),
                                           (0, 2, 1))).reshape(B * C, T)
    shards = []
    for core in range(NCORES):
        seqs = xs[core * S: (core + 1) * S]
        Xm = np.ascontiguousarray(
            seqs.reshape(S, N, L).transpose(2, 1, 0).astype(np.float16)
        ).reshape(L, N * S)
        shards.append(Xm)
    return shards


def _unshard_output(outs):
    """list of per-core [128, T] device outputs -> [B, T, C] fp32."""
    ys = np.empty((B * C, T), dtype=np.float32)
    for core in range(NCORES):
        O = np.asarray(outs[core]).astype(np.float32)
        ys[core * S: (core + 1) * S] = (
            O.reshape(L, N, S).transpose(2, 1, 0).reshape(S, T))
    return np.ascontiguousarray(ys.reshape(B, C, T).transpose(0, 2, 1))


def kernel(x, bp_sos, lp_sos, _trace=False):
    from concourse.bass_utils import run_bass_kernel_spmd

    (t0bp, t1bp), trunc_bp = _filter_weights(np.asarray(bp_sos))
    (t0lp, t1lp), trunc_lp = _filter_weights(np.asarray(lp_sos))
    assert max(trunc_bp, trunc_lp) < 1e-5, (
        "FIR truncation invalid for these coefficients", trunc_bp, trunc_lp)
    consts = {"t0bp": t0bp, "t1bp": t1bp, "t0lp": t0lp, "t1lp": t1lp}
    shards = _shard_inputs(x)
    nc = _get_program()
    in_maps = [dict(consts, x=shards[core]) for core in range(NCORES)]
    res = run_bass_kernel_spmd(nc, in_maps, list(range(NCORES)), trace=_trace)
    out = _unshard_output([res.results[core]["out"] for core in range(NCORES)])
    if _trace:
        return out, res
    return out


if __name__ == "__main__":
    rng = np.random.default_rng(0)
    x = rng.standard_normal((B, T, C), dtype=np.float32)
    print("smoke: shard/unshard roundtrip")
    sh = _shard_inputs(x)
    rt = _unshard_output([s for s in sh])
    print("roundtrip close:", np.allclose(rt, x, atol=1e-2))
